# revision 66
# baseline (speedup 1.0000x reference)
"""Trainium2 Bass kernel for windowed (sparse) gated attention.

Problem (hardcoded): B=2, S=4096, D=128, DI=1024 (8 heads x 128), W=128.
For each query window i (of 32), keys/values come from windows i-1,i,i+1
(3W=384 keys, zero-padded at sequence edges), plus an additive [S,S] bias
read only on those diagonal bands; softmax; gated by sigmoid(x@Wg.T+bg);
output projection Wo.

Sharding: sequence-parallel. Core c owns query windows [4c, 4c+4) for both
batches / all heads; it receives a halo'd, pre-transposed fp16 slice of seq
and per-head fp16 multipliers exp(bias + x_k.c_h) for the 12 valid
(key-window, query-window) blocks, so there is no inter-core communication.
Output is returned transposed per core ([B, 128, 512]) and re-assembled on
the host.

Device scheme (all matmul operands fp16, PSUM f32):
- The Q projection is folded into K on the host: M_h = Wk_h^T Wq_h/sqrt(dh),
  u = M_h^T x, so scoresT[k,q] = u_h^T x directly (no Q matmuls or q-bias
  adds on device). The q-bias key term exp(x_k.c_h), c_h = Wk_h^T bq_h
  /sqrt(dh), is folded into the host-shipped exp(bias) multiplier.
- Scores are computed transposed in tight per-key-window bands (widths
  128/256/384/384/256/128 queries) packed two-bands-per-PSUM-bank, so one
  Exp activation covers a 2-bank half via a strided AP; attn = exp(s) *
  ebias via a single 2x-mode fp16 tensor_tensor per half (ebias is 0 on
  masked/invalid positions, implementing the mask and band edges exactly).
- Softmax denominators: ones-column stationary matmuls accumulate per-pair
  row sums (partitions 0/32); one fast reciprocal per head pair; the
  unnormalized gated output og = (tanh_g+1)*attn_out is written as soon as
  AV finishes (freeing PSUM), then normalized by the partition-broadcast
  reciprocal (GPSIMD convert+broadcast mid-stream, PE ones-row broadcast on
  the batch tail). Sigmoid's 0.5 is host-folded into Wo.
- Emission is software-pipelined: scores/exp run one head ahead, u/gate
  projections two ahead, reciprocal tails are deferred one pair to keep the
  DVE queue clear, and batch 1's projections fill batch 0's wind-down.
"""

import numpy as np

import concourse.bass as bass
import concourse.mybir as mybir
import concourse.tile as tile
from concourse import bacc

F32 = mybir.dt.float32
F16 = mybir.dt.float16

B, S, D, DI, W, H, DH = 2, 4096, 128, 1024, 128, 8, 128
NCORES = 8
NWIN = S // W                 # 32 windows total
NW = NWIN // NCORES           # 4 query windows per core
SC = NW * W                   # 512 query positions per core
NJ = NW + 2                   # 6 key windows per core (with halo)
SL = NJ * W                   # 768 key positions per core

# ---------------------------------------------------------------- config
CFG = dict(
    use_f32r=False,       # legacy flag (test.py pokes it); fp16 path ignores it
    u_copy_on_act=True,   # u (folded-QK) PSUM->SBUF copies on ACT (else DVE)
    vv_on_act=False,      # v PSUM->SBUF copies on ACT (else DVE)
    ognorm_on_pool=True,  # og normalize tensor_tensor on GPSIMD (else DVE)
    nrep=1,
)

# band geometry, key window J = Jl - 1 for Jl in 0..5
_WMIN = [max(Jl - 2, 0) for Jl in range(6)]          # first q window of band
_WMAX = [min(Jl, 3) for Jl in range(6)]              # last q window of band
_NKW = [_WMAX[j] - _WMIN[j] + 1 for j in range(6)]   # band width in windows
_POS0 = [0, 1, 3, 6, 9, 11]                          # attnT slot of band start


def _pos(Jl, w):
    return _POS0[Jl] + (w - _WMIN[Jl])


# score PSUM placement: per half, flat (elem) offsets inside a [128,2,512]
# tile; bands are packed [0:384] in bank0 and [512:896] in bank1 so the Exp
# reads ph[:, :, 0:384] as one strided AP.
_HALF_BANDS = [
    [(0, 0), (1, 128), (2, 512)],     # (Jl, flat offset): J=-1, J=0 | J=1
    [(3, 0), (4, 512), (5, 768)],     # J=2 | J=3, J=4
]


# ---------------------------------------------------------------- device
def _build_device(nc, t):
    AF = mybir.ActivationFunctionType
    ALU = mybir.AluOpType

    from contextlib import ExitStack

    with tile.TileContext(nc) as tc, ExitStack() as st:
        cpool = st.enter_context(tc.tile_pool(name="consts", bufs=1))
        wpool = st.enter_context(tc.tile_pool(name="weights", bufs=1))
        bpool = st.enter_context(tc.tile_pool(name="batch", bufs=2))
        apool = st.enter_context(tc.tile_pool(name="attn", bufs=2))
        ypool = st.enter_context(tc.tile_pool(name="yout", bufs=2))
        ps = st.enter_context(tc.tile_pool(name="ps", bufs=2, space="PSUM"))

        # ---- input + weight DMAs (SP engine), first-use order
        x0 = bpool.tile([128, SL], F16, tag="x", name="x0")
        nc.sync.dma_start(x0, t["xT"][0])
        wv = wpool.tile([128, DI], F16, tag="wv")
        nc.sync.dma_start(wv, t["wvT"][:])
        mm8 = wpool.tile([128, H, 128], F16, tag="mm8")
        nc.sync.dma_start(mm8, t["mm8"][:])
        wg = wpool.tile([128, DI], F16, tag="wg")
        nc.sync.dma_start(wg, t["wgT"][:])
        bg8 = wpool.tile([128, 8], F32, tag="bg8")
        nc.sync.dma_start(bg8, t["bg8"][:])
        # per-(batch, head) exp-bias tiles (f-factor folded in on the host);
        # separate DMAs so early heads' slices land first
        ebs = [[], []]
        for h in range(H):
            e = bpool.tile([128, 12, 128], F16, tag="eb8", bufs=16,
                           name=f"eb0_{h}")
            nc.sync.dma_start(e, t["eb8"][0, h])
            ebs[0].append(e)
        x1 = bpool.tile([128, SL], F16, tag="x", name="x1")
        nc.sync.dma_start(x1, t["xT"][1])
        wo = wpool.tile([128, H, 128], F16, tag="wo")
        nc.sync.dma_start(wo, t["woT"][:])
        for h in range(H):
            e = bpool.tile([128, 12, 128], F16, tag="eb8", bufs=16,
                           name=f"eb1_{h}")
            nc.sync.dma_start(e, t["eb8"][1, h])
            ebs[1].append(e)
        xs = [x0, x1]
        ones_col = cpool.tile([128, 1], F16, tag="ones_col")
        nc.gpsimd.memset(ones_col[:], 1.0)
        ones_row = cpool.tile([1, 128], F16, tag="ones_row")
        nc.gpsimd.memset(ones_row[:], 1.0)

        for rep in range(CFG["nrep"]):
          S_ = [dict() for _ in range(B)]

          def alloc_batch(b):
            st = S_[b]
            st["uT"] = bpool.tile([128, H, SL], F16, tag="uT",
                                  name=f"uT{b}_{rep}")
            st["gT"] = bpool.tile([128, H, SC], F16, tag="gT",
                                  name=f"gT{b}_{rep}")
            st["og"] = apool.tile([128, H, SC], F16, tag="og",
                                  name=f"og{b}_{rep}")

          def proj_V(b):
            x = xs[b]
            st = S_[b]
            st["vv"] = vv = bpool.tile([128, NJ, DI], F16, tag="vv",
                                       name=f"vv{b}_{rep}")
            for j in range(NJ):
                xj = x[:, j * 128 : (j + 1) * 128]
                pv = ps.tile([128, 2, 512], F32, tag="p2",
                             name=f"pv{b}{j}_{rep}")
                nc.tensor.matmul(pv[:, 0, :], xj, wv[:, 0:512],
                                 start=True, stop=True)
                nc.tensor.matmul(pv[:, 1, :], xj, wv[:, 512:1024],
                                 start=True, stop=True)
                pvf = pv.rearrange("p a b -> p (a b)")
                if j % 2 == 0:
                    nc.scalar.copy(vv[:, j, :], pvf[:, 0:DI])
                else:
                    nc.vector.tensor_copy(vv[:, j, :], pvf[:, 0:DI])
          def ug_chunk(b, c):
            """u (folded QK) + sigmoid-gate projections for head-chunk c."""
            x, st = xs[b], S_[b]
            pu = ps.tile([128, 2, 512], F32, tag="p2", name=f"pu{b}{c}_{rep}")
            nc.tensor.matmul(pu[:, 0, :], mm8[:, c, :], x[:, 0:512],
                             start=True, stop=True)
            nc.tensor.matmul(pu[:, 1, 0:256], mm8[:, c, :], x[:, 512:768],
                             start=True, stop=True)
            puf = pu.rearrange("p a b -> p (a b)")
            if c % 2 == 0:
                nc.scalar.copy(st["uT"][:, c, :], puf[:, 0:SL])
            else:
                nc.vector.tensor_copy(st["uT"][:, c, :], puf[:, 0:SL])

            pg = ps.tile([128, 512], F32, tag="pb", name=f"pg{b}{c}_{rep}")
            nc.tensor.matmul(pg, wg[:, c * 128 : (c + 1) * 128],
                             x[:, W : W + SC], start=True, stop=True)
            # sigmoid via tanh (same ACT table set as Exp - a set switch
            # costs a 1283ns LoadActFuncSet): sig = 0.5*(tanh(z/2)+1);
            # +1 via 4x-mode scalar add, 0.5 folded into Wo on the host
            nc.scalar.activation(st["gT"][:, c, :], pg, AF.Tanh,
                                 bias=bg8[:, c : c + 1], scale=0.5)

          def scores_exp_mult(b, h):
            """scores -> exp -> (f*exp(bias)) rescale, per half, for head h."""
            x, st = xs[b], S_[b]
            attR = apool.tile([128, 12, 128], F16, tag="attR", bufs=3,
                              name=f"attR{b}{h}_{rep}")
            attnT = apool.tile([128, 12, 128], F16, tag="attnT", bufs=3,
                               name=f"attnT{b}{h}_{rep}")
            for half in range(2):
                ph = ps.tile([128, 2, 512], F32, tag="p2",
                             name=f"ph{b}{h}{half}_{rep}")
                phf = ph.rearrange("p a b -> p (a b)")
                for Jl, off in _HALF_BANDS[half]:
                    width = _NKW[Jl] * 128
                    qlo = _WMIN[Jl] * 128
                    nc.tensor.matmul(
                        phf[:, off : off + width],
                        st["uT"][:, h, Jl * 128 : (Jl + 1) * 128],
                        x[:, W + qlo : W + qlo + width],
                        start=True, stop=True,
                    )
                sl = slice(half * 6, (half + 1) * 6)
                nc.scalar.activation(attR[:, sl, :], ph[:, :, 0:384], AF.Exp)
                nc.vector.tensor_tensor(
                    attnT[:, sl, :].rearrange("p a b -> p (a b)"),
                    attR[:, sl, :].rearrange("p a b -> p (a b)"),
                    ebs[b][h][:, sl, :].rearrange("p a b -> p (a b)"),
                    ALU.mult,
                )
            return attnT

          def head_step(b, h):
            st = S_[b]
            attnT = st["attnT"]
            if h % 2 == 0:
                st["psums"] = ps.tile([128, 512], F32, tag="pa",
                                      name=f"psums{b}{h}_{rep}")
            psums = st["psums"]
            prow = 32 * (h % 2)
            for w in range(NW):
                for i in range(3):
                    nc.tensor.matmul(
                        psums[prow : prow + 1, w * 128 : (w + 1) * 128],
                        ones_col, attnT[:, _pos(w + i, w), :],
                        start=(i == 0), stop=(i == 2),
                    )
            if h == H - 1:
                # last head of the batch: emit the pair reciprocal right
                # after its sums so it overlaps the AV matmuls below
                st["rinv_last"] = _recip(b, h, psums)
            poT = ps.tile([128, 512], F32, tag="pb", name=f"poT{b}{h}_{rep}")
            for w in range(NW):
                for i in range(3):
                    Jl = w + i
                    nc.tensor.matmul(
                        poT[:, w * 128 : (w + 1) * 128],
                        st["vv"][:, Jl, h * 128 : (h + 1) * 128],
                        attnT[:, _pos(Jl, w), :],
                        start=(i == 0), stop=(i == 2),
                    )
            # unnormalized gated output right away - frees the PSUM slot
            # without waiting for the reciprocal chain; gate = tanh + 1
            # (sigmoid identity; the 0.5 is host-folded into Wo)
            og = st["og"]
            nc.vector.scalar_tensor_tensor(
                og[:, h, :], st["gT"][:, h, :], 1.0, poT, ALU.add, ALU.mult
            )

          def _recip(b, h, psums):
            # one reciprocal per head pair (rows at partitions 0 and 32;
            # the rows between them are unused)
            rinv = apool.tile([33, 512], F32, tag="rinv", bufs=4,
                              name=f"rinv_{b}{h}_{rep}")
            with nc.allow_low_precision(reason="softmax recip"):
                nc.vector.reciprocal_approx_fast(rinv, psums[0:33, :])
            return rinv

          def _norm_one(b, hh, rinv):
            # fp16 convert + partition broadcast on GPSIMD, then normalize og
            og = S_[b]["og"]
            r16 = apool.tile([1, 512], F16, tag="r16", bufs=4,
                             name=f"r16_{b}{hh}_{rep}")
            pr = 32 * (hh % 2)
            if b == B - 1 and hh >= 4:
                # last batch's late pairs: all exps are already emitted, the
                # otherwise-idle ACT takes the convert off the GPSIMD chain
                nc.scalar.copy(r16, rinv[pr : pr + 1, :])
            else:
                nc.gpsimd.tensor_copy(r16, rinv[pr : pr + 1, :])
            bc = apool.tile([128, 512], F16, tag="bc", bufs=4,
                            name=f"bc{b}{hh}_{rep}")
            nc.gpsimd.partition_broadcast(bc, r16)
            if CFG["ognorm_on_pool"]:
                nc.gpsimd.tensor_tensor(og[:, hh, :], og[:, hh, :], bc,
                                        ALU.mult)
            else:
                nc.vector.tensor_tensor(og[:, hh, :], og[:, hh, :], bc,
                                        ALU.mult)

          def pair_tail(b, h, psums):
            rinv = _recip(b, h, psums)
            _norm_one(b, h - 1, rinv)
            _norm_one(b, h, rinv)

          def _norm_one_pe(b, hh, rinv):
            # tail variant: broadcast via a PE ones-row matmul (PE/DVE are
            # idle at the batch tail; skips the serial GPSIMD chain)
            og = S_[b]["og"]
            r16 = apool.tile([1, 512], F16, tag="r16", bufs=4,
                             name=f"r16_{b}{hh}_{rep}")
            pr = 32 * (hh % 2)
            if b == B - 1:
                nc.scalar.copy(r16, rinv[pr : pr + 1, :])  # ACT idle at end
            else:
                nc.vector.tensor_copy(r16, rinv[pr : pr + 1, :])
            prb = ps.tile([128, 512], F32, tag="pb", name=f"prb{b}{hh}_{rep}")
            nc.tensor.matmul(prb, ones_row, r16, start=True, stop=True)
            nc.vector.tensor_tensor(og[:, hh, :], og[:, hh, :], prb, ALU.mult)

          def head_loop(b, tail=None, tail_at=H - 3):
            if "attnT" not in S_[b]:
                S_[b]["attnT"] = scores_exp_mult(b, 0)
            pending = []   # pair tails, deferred one pair to keep the DVE
            for h in range(H):   # queue clear ahead of critical rescales
                attnT_next = scores_exp_mult(b, h + 1) if h + 1 < H else None
                if h + 2 < H:
                    ug_chunk(b, h + 2)
                if h == H - 1 and pending:
                    # drain the previous pair before the last head so its
                    # GPSIMD chain overlaps the final head's matmuls
                    ph_, pp_ = pending.pop(0)
                    pair_tail(b, ph_, pp_)
                head_step(b, h)
                if h % 2 == 1 and h < H - 1:
                    pending.append((h, S_[b]["psums"]))
                    if len(pending) > 1:
                        ph_, pp_ = pending.pop(0)
                        pair_tail(b, ph_, pp_)
                S_[b]["attnT"] = attnT_next
                if tail is not None and h == tail_at:
                    # fill this batch's wind-down (or ramp-up) with the
                    # other batch's work
                    tail()
            # last pair: reciprocal was emitted inside head_step(H-1)
            rinv = S_[b]["rinv_last"]
            _norm_one_pe(b, H - 2, rinv)
            pf = ps.tile([128, 512], F32, tag="pa", name=f"pf{b}_{rep}")
            S_[b]["pf"] = pf
            for c in range(H - 2):
                nc.tensor.matmul(pf, wo[:, c, :], S_[b]["og"][:, c, :],
                                 start=(c == 0), stop=False)
            _norm_one_pe(b, H - 1, rinv)

          def final(b):
            pf = S_[b]["pf"]
            for c in range(H - 2, H):
                nc.tensor.matmul(pf, wo[:, c, :], S_[b]["og"][:, c, :],
                                 start=False, stop=(c == H - 1))
            y = ypool.tile([128, 512], F32, tag="y", name=f"y{b}_{rep}")
            nc.scalar.copy(y, pf)
            nc.sync.dma_start(t["yT"][b], y)

          def start_next_batch():
            alloc_batch(1)
            ug_chunk(1, 0)
            ug_chunk(1, 1)
            proj_V(1)

          alloc_batch(0)
          ug_chunk(0, 0)
          ug_chunk(0, 1)
          S_[0]["attnT"] = scores_exp_mult(0, 0)
          proj_V(0)
          head_loop(0, tail=start_next_batch)
          # batch 1's first scores run on the PE while batch 0's last
          # normalizations drain on DVE/GPSIMD
          S_[1]["attnT"] = scores_exp_mult(1, 0)
          final(0)
          head_loop(1)
          final(1)


# ---------------------------------------------------------------- build
_CACHE = {}


def _get_nc():
    key = tuple(sorted((k, v) for k, v in CFG.items()))
    if _CACHE.get("key") == key:
        return _CACHE["nc"], _CACHE["t"]
    nc = bacc.Bacc(None, target_bir_lowering=False)
    t = dict(
        xT=nc.dram_tensor("xT", [B, 128, SL], F16, kind="ExternalInput"),
        eb8=nc.dram_tensor("eb8", [B, H, 128, 12, 128], F16,
                           kind="ExternalInput"),
        mm8=nc.dram_tensor("mm8", [128, H, 128], F16, kind="ExternalInput"),
        wvT=nc.dram_tensor("wvT", [128, DI], F16, kind="ExternalInput"),
        wgT=nc.dram_tensor("wgT", [128, DI], F16, kind="ExternalInput"),
        woT=nc.dram_tensor("woT", [128, H, 128], F16, kind="ExternalInput"),
        bg8=nc.dram_tensor("bg8", [128, 8], F32, kind="ExternalInput"),
        yT=nc.dram_tensor("yT", [B, 128, SC], F32, kind="ExternalOutput"),
    )
    _build_device(nc, t)
    nc.compile()
    _CACHE["nc"], _CACHE["t"], _CACHE["key"] = nc, t, key
    return nc, t


# ---------------------------------------------------------------- host
def _prep_shared(Wq, bq, Wkv, Wg, bg, Wo):
    scale = DH ** -0.5
    Wq = np.asarray(Wq, np.float32).reshape(H, DH, D)
    bqh = np.asarray(bq, np.float32).reshape(H, DH)
    Wk = np.asarray(Wkv, np.float32)[:DI].reshape(H, DH, D)
    # M_h = Wk_h^T Wq_h * scale  as [d1, h, d2];  c_h = Wk_h^T bq_h * scale
    M = np.einsum("hki,hkj->ihj", Wk, Wq) * scale
    c = np.einsum("hki,hk->ih", Wk, bqh) * scale
    wvT = np.ascontiguousarray(np.asarray(Wkv, np.float32)[DI:].T)
    wgT = np.ascontiguousarray(np.asarray(Wg, np.float32).T)
    woT = np.ascontiguousarray(
        (0.5 * np.asarray(Wo, np.float32)).T.reshape(H, 128, 128)
        .transpose(1, 0, 2)
    )
    bg8 = np.ascontiguousarray(
        (np.asarray(bg, np.float32) * 0.5).reshape(H, 128).T
    )
    return dict(
        mm8=np.ascontiguousarray(M, np.float16),
        wvT=wvT.astype(np.float16),
        wgT=wgT.astype(np.float16),
        woT=woT.astype(np.float16),
        bg8=bg8,
    ), c


_POS2JL = [0, 1, 1, 2, 2, 2, 3, 3, 3, 4, 4, 5]


def _prep_core(ci, seq, attn_bias, cvec):
    lo = ci * SC - W
    hi = ci * SC + SC + W
    xs = np.zeros((B, SL, D), np.float32)
    a, bnd = max(lo, 0), min(hi, S)
    xs[:, a - lo : bnd - lo, :] = seq[:, a:bnd, :]
    xT = np.ascontiguousarray(xs.transpose(0, 2, 1)).astype(np.float16)

    br = attn_bias.reshape(B, NWIN, W, NWIN, W)
    eb = np.zeros((B, 12, W, W), np.float32)      # [b, pos, k, q]
    for Jl in range(6):
        gk = 4 * ci + Jl - 1                       # global key window
        for w in range(_WMIN[Jl], _WMAX[Jl] + 1):
            if 0 <= gk < NWIN:
                gq = 4 * ci + w
                eb[:, _pos(Jl, w)] = np.exp(
                    br[:, gq, :, gk, :]
                ).transpose(0, 2, 1)
    # fold the q-bias key factor f = exp(x_k . c_h) in per head:
    # eb8[b, h, k, pos, q] = eb[b, pos, k, q] * f[b, Jl(pos)*128 + k, h]
    f = np.exp(xs @ cvec).reshape(B, NJ, W, H)     # [b, Jl, k, h]
    fpos = f[:, _POS2JL]                           # [b, pos, k, h]
    eb8 = eb.transpose(0, 2, 1, 3)[:, None] * fpos.transpose(0, 3, 2, 1)[
        :, :, :, :, None
    ]                                              # [b, h, k, pos, q]
    return dict(xT=xT, eb8=np.ascontiguousarray(eb8, np.float16))


def kernel(seq, mask, attn_bias, Wq, bq, Wkv, Wg, bg, Wo):
    from concourse.bass_utils import run_bass_kernel_spmd

    nc, _ = _get_nc()
    seq = np.asarray(seq, np.float32)
    attn_bias = np.asarray(attn_bias, np.float32)
    shared, cvec = _prep_shared(Wq, bq, Wkv, Wg, bg, Wo)
    in_maps = []
    for ci in range(NCORES):
        in_maps.append(dict(_prep_core(ci, seq, attn_bias, cvec), **shared))

    res = run_bass_kernel_spmd(nc, in_maps, core_ids=list(range(NCORES)))
    y = np.empty((B, S, D), np.float32)
    for c in range(NCORES):
        yT = res.results[c]["yT"]                                    # [B,128,512]
        y[:, c * SC : (c + 1) * SC, :] = yT.transpose(0, 2, 1)
    return y
